# revision 8
# baseline (speedup 1.0000x reference)
"""Trainium2 Bass kernel for the 2-layer GAT (nn_GNN_5952824672568).

Strategy (8 NeuronCores, dst-sharded graph parallel):
  - Host: shard nodes 2500/core, permute each core's nodes into 20 degree-balanced
    dst-blocks of 128 slots (2560 padded slots/core), sort edges by dst-block,
    pad each block's edge list to T tiles of 128 edges (uniform across cores).
  - Device, per GAT layer:
      node phase: xl = x @ W (PE, lhsT = x^T slices), attention dots via
        host-premultiplied (W @ a) matrices sharing the same stationary lhsT,
        exp() at node level only:  exp(leaky(as+ad)) == max(ES*ED, ES2*ED2)
        with ES=exp(as), ES2=exp(.2 as), ED=exp(ad), ED2=exp(.2 ad).
        Write bf16 table rows [xl | ES | ES2] -> AllGather full 20480x520 table.
      edge phase: per dst-block (20) x edge-tile (17): indirect-DMA gather 128
        table rows by src, expand dst terms with a host-built one-hot S_T via a
        tiny matmul, DVE computes per-edge exp weights and weighted features,
        then a one-hot scatter matmul accumulates both the weighted feature sum
        and the softmax denominator into PSUM. Divide once per block.
      BN: masked partial sums via matmul, AllReduce, affine+LeakyReLU on DVE/ACT.
  - Residual is folded into the final linear's PSUM accumulation.
  - Host: unpermute rows of the per-core outputs.
"""
import sys
for _p in ("/opt/trn_rl_repo", "/root/.axon_site/_ro/trn_rl_repo"):
    if _p not in sys.path:
        sys.path.append(_p)

import numpy as np
import ml_dtypes

N, E, F_IN = 20000, 320000, 127
H, C = 4, 128
D = H * C
OUT = 256
P = 128
NCORES = 8
NL = N // NCORES            # 2500 owned nodes per core
NBLK = (NL + P - 1) // P    # 20 dst blocks
NP = NBLK * P               # 2560 padded local slots
TW = D + 2 * H              # 520 table row width
NEG_ATT, NEG_ACT = 0.2, 0.01
BN_EPS = 1e-5

_bf16 = ml_dtypes.bfloat16
_CACHE = {}


# ----------------------------------------------------------------- host prep
def _preprocess(edge_index):
    src = np.concatenate([edge_index[0], np.arange(N)]).astype(np.int64)
    dst = np.concatenate([edge_index[1], np.arange(N)]).astype(np.int64)
    cores = []
    for k in range(NCORES):
        lo = k * NL
        m = (dst >= lo) & (dst < lo + NL)
        es, ed = src[m], dst[m] - lo
        deg = np.bincount(ed, minlength=NL)
        order = np.argsort(-deg, kind="stable")
        new_of = np.empty(NL, np.int64)
        ar = np.arange(NL)
        new_of[order] = (ar % NBLK) * P + (ar // NBLK)
        ed2 = new_of[ed]
        o = np.argsort(ed2, kind="stable")
        cores.append((es[o], ed2[o], new_of))

    T = 0
    for es, ed2, _ in cores:
        cnt = np.bincount(ed2 // P, minlength=NBLK)
        T = max(T, int(np.ceil(cnt.max() / P)))

    row_of = np.empty(N, np.int64)
    for k in range(NCORES):
        row_of[np.arange(k * NL, (k + 1) * NL)] = k * NP + cores[k][2]

    metas, sts, masks = [], [], []
    for k, (es, ed2, new_of) in enumerate(cores):
        meta = np.zeros((NBLK, T, P, 2), np.int32)
        meta[:, :, :, 1] = 2 * P  # sentinel dst_local: matches no one-hot column
        st = np.zeros((NBLK, T, P, P), _bf16)
        for b in range(NBLK):
            m = (ed2 // P) == b
            gs, gd = row_of[es[m]], ed2[m] % P
            n_e = len(gs)
            fl = np.zeros(T * P, np.int64)
            fd = np.full(T * P, 2 * P, np.int64)
            fl[:n_e] = gs
            fd[:n_e] = gd
            meta[b, :, :, 0] = fl.reshape(T, P)
            meta[b, :, :, 1] = fd.reshape(T, P).astype(np.float32).view(np.int32)
            oh = (fd.reshape(T, P)[:, None, :] == np.arange(P)[None, :, None])
            st[b] = oh.astype(_bf16)  # [T, 128d, 128e]
        mk = np.zeros(NP, np.float32)
        mk[new_of] = 1.0
        metas.append(meta)
        sts.append(st)
        masks.append(mk.reshape(NBLK, P).T.copy())  # [128, NBLK]
    return cores, metas, sts, masks, T


def _premul_a(W, a_s, a_d):
    """WA[j, h]   = sum_c W[j, h*C+c] * a[h, c]  for a_s (cols 0:4) and a_d (4:8)."""
    Wr = W.reshape(W.shape[0], H, C)
    return np.concatenate(
        [np.einsum("jhc,hc->jh", Wr, a_s), np.einsum("jhc,hc->jh", Wr, a_d)], 1
    ).astype(np.float32)


# --------------------------------------------------------------- bass program
def _build_program(T):
    import concourse.bass as bass
    import concourse.bacc as bacc
    import concourse.mybir as mybir
    import concourse.tile as tile
    from concourse.masks import make_identity
    fp32, bf16, i32 = mybir.dt.float32, mybir.dt.bfloat16, mybir.dt.int32

    nc = bacc.Bacc("TRN2", target_bir_lowering=False, debug=False,
                   num_devices=NCORES)

    xT = nc.dram_tensor("xT", [P, NP], bf16, kind="ExternalInput")
    meta_d = nc.dram_tensor("meta", [NBLK, T, P, 2], i32, kind="ExternalInput")
    st_d = nc.dram_tensor("st", [NBLK, T, P, P], bf16, kind="ExternalInput")
    maskT_d = nc.dram_tensor("maskT", [P, NBLK], fp32, kind="ExternalInput")
    colids_d = nc.dram_tensor("colids", [P, P], fp32, kind="ExternalInput")
    W1_d = nc.dram_tensor("W1", [P, D], bf16, kind="ExternalInput")
    WA1_d = nc.dram_tensor("WA1", [P, 2 * H], bf16, kind="ExternalInput")
    W2_d = nc.dram_tensor("W2", [4, P, D], bf16, kind="ExternalInput")
    WA2_d = nc.dram_tensor("WA2", [4, P, 2 * H], bf16, kind="ExternalInput")
    OW_d = nc.dram_tensor("OW", [4, P, OUT], bf16, kind="ExternalInput")
    bnp_d = nc.dram_tensor("bnp", [1, 4 * D], fp32, kind="ExternalInput")
    outb_d = nc.dram_tensor("outb", [1, OUT], fp32, kind="ExternalInput")
    out_d = nc.dram_tensor("out", [NP, OUT], fp32, kind="ExternalOutput")

    rg = [list(range(NCORES))]
    AF = mybir.ActivationFunctionType
    ALU = mybir.AluOpType

    with tile.TileContext(nc) as tc:
        with tc.tile_pool(name="const", bufs=1) as cp, \
             tc.tile_pool(name="dram", bufs=1, space="DRAM") as dp, \
             tc.tile_pool(name="sb", bufs=3) as sp, \
             tc.tile_pool(name="big", bufs=1) as bigp, \
             tc.tile_pool(name="ps", bufs=2, space="PSUM") as pp, \
             tc.tile_pool(name="ps1", bufs=1, space="PSUM") as pp1:

            # ---- persistent SBUF state
            colids = cp.tile([P, P], fp32)
            nc.sync.dma_start(out=colids[:], in_=colids_d[:])
            maskT = cp.tile([P, NBLK], fp32)
            nc.sync.dma_start(out=maskT[:], in_=maskT_d[:])
            xTs = cp.tile([P, NP], bf16)
            nc.sync.dma_start(out=xTs[:], in_=xT[:])
            W1s = cp.tile([P, D], bf16)
            nc.sync.dma_start(out=W1s[:], in_=W1_d[:])
            WA1s = cp.tile([P, 2 * H], bf16)
            nc.sync.dma_start(out=WA1s[:], in_=WA1_d[:])
            W2s = cp.tile([P, 4 * D], bf16)
            for g in range(4):
                nc.sync.dma_start(out=W2s[:, g * D:(g + 1) * D], in_=W2_d[g])
            WA2s = cp.tile([P, 4 * 2 * H], bf16)
            for g in range(4):
                nc.sync.dma_start(out=WA2s[:, g * 8:(g + 1) * 8], in_=WA2_d[g])
            OWs = cp.tile([P, 4 * OUT], bf16)
            for g in range(4):
                nc.sync.dma_start(out=OWs[:, g * OUT:(g + 1) * OUT], in_=OW_d[g])
            bnps = cp.tile([1, 4 * D], fp32)
            nc.sync.dma_start(out=bnps[:], in_=bnp_d[:])
            outbs = cp.tile([1, OUT], fp32)
            nc.sync.dma_start(out=outbs[:], in_=outb_d[:])

            eds1 = cp.tile([P, 8 * NBLK], bf16, tag="eds1")
            eds2 = cp.tile([P, 8 * NBLK], bf16, tag="eds2")
            h1 = bigp.tile([P, NBLK * D], fp32, tag="h1")       # 5.2 MB
            h2 = None  # allocated later, reuses h1 slot (h1 dead after transpose)
            hT1 = bigp.tile([P, 4 * NBLK * P], bf16, tag="hT1")  # 2.6 MB
            hT2 = bigp.tile([P, 4 * NBLK * P], bf16, tag="hT2")
            arep = cp.tile([P, D], fp32, tag="arep")
            crep = cp.tile([P, D], fp32, tag="crep")
            obrep = cp.tile([P, OUT], fp32, tag="obrep")
            nc.gpsimd.partition_broadcast(obrep[:], outbs[:])

            # ---- DRAM tiles
            tab_loc = dp.tile([NP, TW], bf16, tag="tabloc")
            table = dp.tile([NCORES * NP, TW], bf16, tag="table", addr_space="Shared")
            tab_loc2 = dp.tile([NP, TW], bf16, tag="tabloc2")
            table2 = dp.tile([NCORES * NP, TW], bf16, tag="table2", addr_space="Shared")
            bnb_in = dp.tile([1, 2 * D], fp32, tag="bnbin")
            bnb_out = dp.tile([1, 2 * D], fp32, tag="bnbout", addr_space="Shared")
            bnb_in2 = dp.tile([1, 2 * D], fp32, tag="bnbin2")
            bnb_out2 = dp.tile([1, 2 * D], fp32, tag="bnbout2", addr_space="Shared")

            def node_phase(layer, eds, tl):
                """Compute xl, attention exps; write bf16 table rows to tl."""
                for b in range(NBLK):
                    ps_xl = pp.tile([P, D], fp32, tag="big")
                    ps_al = pp.tile([P, 8], fp32, tag="ed")
                    if layer == 0:
                        lhs = xTs[:, b * P:(b + 1) * P]
                        nc.tensor.matmul(out=ps_xl[:], lhsT=lhs, rhs=W1s[:],
                                         start=True, stop=True)
                        nc.tensor.matmul(out=ps_al[:], lhsT=lhs, rhs=WA1s[:],
                                         start=True, stop=True)
                    else:
                        for g in range(4):
                            lhs = hT1[:, (g * NBLK + b) * P:(g * NBLK + b + 1) * P]
                            nc.tensor.matmul(out=ps_xl[:], lhsT=lhs,
                                             rhs=W2s[:, g * D:(g + 1) * D],
                                             start=(g == 0), stop=(g == 3))
                            nc.tensor.matmul(out=ps_al[:], lhsT=lhs,
                                             rhs=WA2s[:, g * 8:(g + 1) * 8],
                                             start=(g == 0), stop=(g == 3))
                    stg = sp.tile([P, TW], bf16, tag="stg")
                    nc.vector.tensor_copy(out=stg[:, :D], in_=ps_xl[:])
                    nc.scalar.activation(stg[:, D:D + H], ps_al[:, 0:H], AF.Exp)
                    nc.scalar.activation(stg[:, D + H:D + 2 * H], ps_al[:, 0:H],
                                         AF.Exp, scale=NEG_ATT)
                    nc.scalar.activation(eds[:, 8 * b:8 * b + H], ps_al[:, H:2 * H],
                                         AF.Exp)
                    nc.scalar.activation(eds[:, 8 * b + H:8 * b + 8],
                                         ps_al[:, H:2 * H], AF.Exp, scale=NEG_ATT)
                    nc.sync.dma_start(out=tl[b * P:(b + 1) * P, :], in_=stg[:])

            def edge_phase(eds, tab, h):
                for b in range(NBLK):
                    ps_out = pp.tile([P, D], fp32, tag="big")
                    ps_den = pp.tile([P, 8], fp32, tag="den")
                    for t in range(T):
                        mt = sp.tile([P, 2], i32, tag="mt")
                        nc.sync.dma_start(out=mt[:], in_=meta_d[b, t])
                        stt = sp.tile([P, P], bf16, tag="stt")
                        nc.sync.dma_start(out=stt[:], in_=st_d[b, t])
                        F = sp.tile([P, TW], bf16, tag="F")
                        nc.gpsimd.indirect_dma_start(
                            out=F[:], out_offset=None, in_=tab[:],
                            in_offset=bass.IndirectOffsetOnAxis(ap=mt[:, 0:1], axis=0))
                        ps_ed = pp.tile([P, 8], fp32, tag="ed")
                        nc.tensor.matmul(out=ps_ed[:], lhsT=stt[:],
                                         rhs=eds[:, 8 * b:8 * b + 8],
                                         start=True, stop=True)
                        m12 = sp.tile([P, 8], fp32, tag="m12")
                        nc.vector.tensor_tensor(out=m12[:], in0=F[:, D:D + 8],
                                                in1=ps_ed[:], op=ALU.mult)
                        expl = sp.tile([P, H], fp32, tag="expl")
                        nc.vector.tensor_tensor(out=expl[:], in0=m12[:, 0:H],
                                                in1=m12[:, H:2 * H], op=ALU.max)
                        explb = sp.tile([P, H], bf16, tag="explb")
                        nc.vector.tensor_copy(out=explb[:], in_=expl[:])
                        S = sp.tile([P, P], bf16, tag="S")
                        nc.vector.tensor_scalar(out=S[:], in0=colids[:],
                                                scalar1=mt[:, 1:2].bitcast(fp32),
                                                scalar2=None, op0=ALU.is_equal)
                        Fp = sp.tile([P, D], bf16, tag="Fp")
                        for hh in range(H):
                            nc.vector.tensor_scalar(
                                out=Fp[:, hh * C:(hh + 1) * C],
                                in0=F[:, hh * C:(hh + 1) * C],
                                scalar1=expl[:, hh:hh + 1], scalar2=None,
                                op0=ALU.mult)
                        nc.tensor.matmul(out=ps_out[:], lhsT=S[:], rhs=Fp[:],
                                         start=(t == 0), stop=(t == T - 1))
                        nc.tensor.matmul(out=ps_den[:, 0:H], lhsT=S[:], rhs=explb[:],
                                         start=(t == 0), stop=(t == T - 1))
                    den = sp.tile([P, H], fp32, tag="dn")
                    nc.vector.tensor_scalar(out=den[:], in0=ps_den[:, 0:H],
                                            scalar1=1e-16, scalar2=None, op0=ALU.add)
                    rden = sp.tile([P, H], fp32, tag="rdn")
                    nc.vector.reciprocal(out=rden[:], in_=den[:])
                    hb = h[:, b * D:(b + 1) * D]
                    for hh in range(H):
                        nc.vector.tensor_scalar(
                            out=hb[:, hh * C:(hh + 1) * C],
                            in0=ps_out[:, hh * C:(hh + 1) * C],
                            scalar1=rden[:, hh:hh + 1], scalar2=None, op0=ALU.mult)

            def bn_phase(h, gi, bi, bnin, bnout):
                """Masked stats -> AllReduce -> affine params in arep/crep."""
                ps_s1 = pp1.tile([1, D], fp32, tag="st1")
                ps_s2 = pp1.tile([1, D], fp32, tag="st2")
                for b in range(NBLK):
                    hb = h[:, b * D:(b + 1) * D]
                    sq = sp.tile([P, D], fp32, tag="sq")
                    nc.scalar.activation(sq[:], hb, AF.Square)
                    mb = maskT[:, b:b + 1]
                    nc.tensor.matmul(out=ps_s1[:], lhsT=mb, rhs=hb,
                                     start=(b == 0), stop=(b == NBLK - 1))
                    nc.tensor.matmul(out=ps_s2[:], lhsT=mb, rhs=sq[:],
                                     start=(b == 0), stop=(b == NBLK - 1))
                stt = sp.tile([1, 2 * D], fp32, tag="bnrow")
                nc.vector.tensor_copy(out=stt[:, :D], in_=ps_s1[:])
                nc.vector.tensor_copy(out=stt[:, D:], in_=ps_s2[:])
                nc.gpsimd.dma_start(out=bnin[:], in_=stt[:])
                nc.gpsimd.collective_compute(
                    "AllReduce", ALU.add, replica_groups=rg,
                    ins=[bnin.opt()], outs=[bnout.opt()])
                gl = sp.tile([1, 2 * D], fp32, tag="bnrow2")
                nc.sync.dma_start(out=gl[:], in_=bnout[:])
                mu = sp.tile([1, D], fp32, tag="mu")
                nc.vector.tensor_scalar(out=mu[:], in0=gl[:, :D], scalar1=1.0 / N,
                                        scalar2=None, op0=ALU.mult)
                var = sp.tile([1, D], fp32, tag="var")
                nc.vector.tensor_tensor(out=var[:], in0=mu[:], in1=mu[:], op=ALU.mult)
                # var = s2/N - mu^2 + eps
                nc.vector.tensor_scalar(out=var[:], in0=var[:], scalar1=-1.0,
                                        scalar2=None, op0=ALU.mult)
                vt = sp.tile([1, D], fp32, tag="vt")
                nc.vector.tensor_scalar(out=vt[:], in0=gl[:, D:], scalar1=1.0 / N,
                                        scalar2=BN_EPS, op0=ALU.mult, op1=ALU.add)
                nc.vector.tensor_tensor(out=var[:], in0=var[:], in1=vt[:], op=ALU.add)
                sd = sp.tile([1, D], fp32, tag="sd")
                nc.scalar.activation(sd[:], var[:], AF.Sqrt)
                rsd = sp.tile([1, D], fp32, tag="rsd")
                nc.vector.reciprocal(out=rsd[:], in_=sd[:])
                arow = sp.tile([1, D], fp32, tag="arow")
                nc.vector.tensor_tensor(out=arow[:], in0=bnps[0:1, gi * D:(gi + 1) * D],
                                        in1=rsd[:], op=ALU.mult)
                crow = sp.tile([1, D], fp32, tag="crow")
                nc.vector.tensor_tensor(out=crow[:], in0=mu[:], in1=arow[:],
                                        op=ALU.mult)
                nc.vector.tensor_tensor(out=crow[:], in0=bnps[0:1, bi * D:(bi + 1) * D],
                                        in1=crow[:], op=ALU.subtract)
                nc.gpsimd.partition_broadcast(arep[:], arow[:])
                nc.gpsimd.partition_broadcast(crep[:], crow[:])

            def affine_lrelu(h):
                for b in range(NBLK):
                    hb = h[:, b * D:(b + 1) * D]
                    nc.vector.tensor_tensor(out=hb, in0=hb, in1=arep[:], op=ALU.mult)
                    nc.vector.tensor_tensor(out=hb, in0=hb, in1=crep[:], op=ALU.add)
                    nc.scalar.activation(hb, hb, AF.Lrelu, alpha=NEG_ACT)

            ident = cp.tile([P, P], fp32, tag="ident")
            make_identity(nc, ident[:])

            def transpose_h(h, hT):
                for b in range(NBLK):
                    for g in range(4):
                        ps_t = pp.tile([P, P], fp32, tag="big")
                        nc.tensor.transpose(
                            out=ps_t[:], in_=h[:, b * D + g * P:b * D + (g + 1) * P],
                            identity=ident[:])
                        nc.vector.tensor_copy(
                            out=hT[:, (g * NBLK + b) * P:(g * NBLK + b + 1) * P],
                            in_=ps_t[:])

            # ------------------------------------------------ pipeline
            node_phase(0, eds1, tab_loc)
            nc.gpsimd.collective_compute(
                "AllGather", ALU.bypass, replica_groups=rg,
                ins=[tab_loc.opt()], outs=[table.opt()])
            edge_phase(eds1, table, h1)
            bn_phase(h1, 0, 1, bnb_in, bnb_out)
            affine_lrelu(h1)
            transpose_h(h1, hT1)

            h2 = bigp.tile([P, NBLK * D], fp32, tag="h1")  # reuse h1 slot
            node_phase(1, eds2, tab_loc2)
            nc.gpsimd.collective_compute(
                "AllGather", ALU.bypass, replica_groups=rg,
                ins=[tab_loc2.opt()], outs=[table2.opt()])
            edge_phase(eds2, table2, h2)
            bn_phase(h2, 2, 3, bnb_in2, bnb_out2)
            affine_lrelu(h2)
            transpose_h(h2, hT2)

            # ---- final linear: out = (h2 + h1) @ out_W + out_b
            for b in range(NBLK):
                ps_o = pp.tile([P, OUT], fp32, tag="big")
                for g in range(4):
                    sl = slice((g * NBLK + b) * P, (g * NBLK + b + 1) * P)
                    nc.tensor.matmul(out=ps_o[:], lhsT=hT1[:, sl],
                                     rhs=OWs[:, g * OUT:(g + 1) * OUT],
                                     start=(g == 0), stop=False)
                    nc.tensor.matmul(out=ps_o[:], lhsT=hT2[:, sl],
                                     rhs=OWs[:, g * OUT:(g + 1) * OUT],
                                     start=False, stop=(g == 3))
                ob = sp.tile([P, OUT], fp32, tag="ob")
                nc.vector.tensor_tensor(out=ob[:], in0=ps_o[:], in1=obrep[:],
                                        op=ALU.add)
                nc.sync.dma_start(out=out_d[b * P:(b + 1) * P, :], in_=ob[:])

    nc.compile()
    return nc


# ------------------------------------------------------------------- runner
def _get_exec(T):
    if T in _CACHE:
        return _CACHE[T]
    nc = _build_program(T)
    _CACHE[T] = nc
    return nc


def _make_in_maps(inp, cores, metas, sts, masks):
    X, pos_emb = inp["X"], inp["pos_emb"]
    x0 = np.concatenate([np.asarray(X, np.float32),
                         np.asarray(pos_emb, np.float32)], 1)
    colids = np.broadcast_to(np.arange(P, dtype=np.float32)[None, :], (P, P)).copy()
    W1, W2, out_W = inp["W1"], inp["W2"], inp["out_W"]
    WA1 = _premul_a(np.asarray(W1, np.float32), np.asarray(inp["a_src1"]),
                    np.asarray(inp["a_dst1"]))
    WA2 = _premul_a(np.asarray(W2, np.float32), np.asarray(inp["a_src2"]),
                    np.asarray(inp["a_dst2"]))
    bnp = np.concatenate([np.asarray(inp["bn1_g"]), np.asarray(inp["bn1_b"]),
                          np.asarray(inp["bn2_g"]), np.asarray(inp["bn2_b"])
                          ]).astype(np.float32).reshape(1, 4 * D)
    W2r = np.asarray(W2, np.float32).reshape(4, P, D).astype(_bf16)
    WA2r = WA2.reshape(4, P, 2 * H).astype(_bf16)
    OWr = np.asarray(out_W, np.float32).reshape(4, P, OUT).astype(_bf16)

    in_maps = []
    for k in range(NCORES):
        new_of = cores[k][2]
        xp = np.zeros((NP, P), np.float32)
        xp[new_of] = x0[k * NL:(k + 1) * NL]
        in_maps.append({
            "xT": np.ascontiguousarray(xp.T).astype(_bf16),
            "meta": metas[k],
            "st": sts[k],
            "maskT": masks[k],
            "colids": colids,
            "W1": np.asarray(W1, np.float32).astype(_bf16),
            "WA1": WA1.astype(_bf16),
            "W2": W2r, "WA2": WA2r, "OW": OWr,
            "bnp": bnp,
            "outb": np.asarray(inp["out_b"], np.float32).reshape(1, OUT),
        })
    return in_maps


def kernel(X, edge_index, edge_weight, pos_emb,
           W1, a_src1, a_dst1, b1, W2, a_src2, a_dst2, b2,
           bn1_g, bn1_b, bn2_g, bn2_b, dm_W, dm_b, out_W, out_b):
    import concourse.bass_utils as bass_utils

    cores, metas, sts, masks, T = _preprocess(np.asarray(edge_index))
    nc = _get_exec(T)
    in_maps = _make_in_maps(dict(X=X, pos_emb=pos_emb, W1=W1, W2=W2, out_W=out_W,
                                 a_src1=a_src1, a_dst1=a_dst1, a_src2=a_src2,
                                 a_dst2=a_dst2, bn1_g=bn1_g, bn1_b=bn1_b,
                                 bn2_g=bn2_g, bn2_b=bn2_b, out_b=out_b),
                            cores, metas, sts, masks)
    res = bass_utils.run_bass_kernel_spmd(nc, in_maps, core_ids=list(range(NCORES)))
    out = np.empty((N, OUT), np.float32)
    for k in range(NCORES):
        out[k * NL:(k + 1) * NL] = res.results[k]["out"][cores[k][2]]
    return out
